# revision 52
# baseline (speedup 1.0000x reference)
"""GATv2 (2-layer) + global-mean-pool + MLP head on 8 Trainium2 NeuronCores.

Self-contained: host preprocessing (numpy) + Bass/Tile program + SPMD run.

Strategy (dst-sharded graph parallel), v5:
  - Nodes/edges sharded across 8 cores by destination-node windows of 128.
  - Layer math folded so each edge needs ONE gathered fp8 table row:
      table row n = S8 * [att*xl[n] (sign-permuted) | att.xl[n]] + [.. | 1 | pad]
    using leaky_relu(z,0.2) = 0.6 z + 0.4 |z|; S8 lifts values into e4m3 range.
  - The per-core table slab IS the local xl product: each core computes
    xl for its own 12544 nodes only and the full fp8 table is assembled by
    an AllGather of slabs (1.66MB/core) -- no replicated table build, no
    full-x input, and no separate h2 exchange between layers.
  - Per-window gathers deposit contiguous [128, c*TW] fp8 tiles; m' psum
    groups of 3 chunks (one 2KB bank): a single fp8 identity matmul streams
    the group's gathered columns, then per-chunk ohT matmuls add xr[dst].
  - exp() without segment-max (validated); padding edges get a -60 exponent
    bias; denominators clamped before reciprocal; single Exp per window on
    the scalar engine (no activation-table thrashing); node-output
    leaky_relu exact on the vector engine (max/min/mad).
  - Attention applied by scaling gathered rows with av (fp8->f16 convert);
    plain one-hot is the pso stationary (fp8 stationary matmuls are slow).
  - Final graph sums via AllReduce of a [128f x 128g] tile; FC head
    replicated.
"""
import numpy as np

P = 128          # partitions / window size / chunk size
TW = 132         # table row width: 128 feats + pl + 1.0 + 2 pad
S8 = 32.0        # fp8 table scale: lifts att-scaled values into e4m3 range

FULL_CFG = dict(N=100000, DIN=64, H=128, E0=600000, G=128, DOUT=16, NCORES=8)

LR_A = 0.495 / 0.505   # |z| coefficient for vector leaky_relu(0.01)
LR_S = 0.505           # scale folded into downstream weights


# ---------------------------------------------------------------------------
# host preprocessing
# ---------------------------------------------------------------------------

def _sign_perm(att):
    pos = np.where(att > 0)[0]
    neg = np.where(att <= 0)[0]
    return np.concatenate([pos, neg]), len(pos)


def _aug_weights(W, b, att, perm, attp):
    H = W.shape[0]
    Wa = np.zeros((H, TW), np.float32)
    ba = np.zeros((TW,), np.float32)
    Wa[:, :H] = W[:, perm] * attp[None, :]
    ba[:H] = b[perm] * attp
    Wa[:, H] = W @ att
    ba[H] = att @ b
    ba[H + 1] = 1.0
    return Wa, ba


def host_prep(inputs, cfg):
    N, DIN, H, E0, G, NCORES = (cfg["N"], cfg["DIN"], cfg["H"], cfg["E0"],
                                cfg["G"], cfg["NCORES"])
    x = np.asarray(inputs["x"], np.float32)
    ei = np.asarray(inputs["edge_index"]).astype(np.int64)
    batch = np.asarray(inputs["batch"]).astype(np.int64)
    get = lambda k: np.asarray(inputs[k], np.float32)
    f16 = np.float16

    NWT = (N + P - 1) // P
    NWC = (NWT + NCORES - 1) // NCORES
    NLOC = NWC * P
    NPAD = NLOC * NCORES
    NT = NPAD // P

    att1, att2 = get("att1"), get("att2")
    perm1, Pp1 = _sign_perm(att1)
    perm2, Pp2 = _sign_perm(att2)
    attp1 = att1[perm1].astype(np.float32)
    attp2 = att2[perm2].astype(np.float32)

    Wl1a, bl1a = _aug_weights(get("Wl1"), get("bl1"), att1, perm1, attp1)
    Wr1a, br1a = _aug_weights(get("Wr1"), get("br1"), att1, perm1, attp1)
    br1a[H + 1] = 0.0
    Wl2a, bl2a = _aug_weights(get("Wl2")[perm1, :], get("bl2"),
                              att2, perm2, attp2)
    Wr2a, br2a = _aug_weights(get("Wr2")[perm1, :], get("br2"),
                              att2, perm2, attp2)
    br2a[H + 1] = 0.0

    # scale feature+pl columns by S8 (fp8 range); the "1" column stays 1.0
    for ba in (bl1a, br1a, bl2a, br2a):
        ba[:H + 1] *= S8

    weights = {
        "wnfc": get("W_nfc").astype(f16),
        "bnfc": get("b_nfc").reshape(H, 1),
        "wl1a": Wl1a.astype(f16), "wr1a": Wr1a.astype(f16),
        "bl1B": np.tile(bl1a, (P, 1)), "br1B": np.tile(br1a, (P, 1)),
        "wl2a": Wl2a.astype(f16), "wr2a": Wr2a.astype(f16),
        "bl2B": np.tile(bl2a, (P, 1)), "br2B": np.tile(br2a, (P, 1)),
        "arec1B": np.tile((1.0 / (attp1 * S8)), (P, 1)),
        "arec2B": np.tile((1.0 / (attp2 * S8)), (P, 1)),
        "bias1B": np.tile(get("bias1")[perm1], (P, 1)),
        "bias2B": np.tile(get("bias2")[perm2], (P, 1)),
        "wfc1": get("W_fc1")[perm2, :].astype(f16),
        "bfc1": get("b_fc1").reshape(1, -1).astype(f16),
        "wfc2": get("W_fc2").astype(f16),
        "bfc2": get("b_fc2").reshape(1, -1).astype(f16),
    }
    assert np.abs(attp1).min() > 1e-12 and np.abs(attp2).min() > 1e-12

    xT = np.zeros((DIN, NPAD), f16)
    xT[:, :N] = x.T.astype(f16)

    # --- edges (self-loops are NOT appended; they become chunk 0/window) ---
    src0, dst0 = ei[0], ei[1]
    per_core = []
    for c in range(NCORES):
        lo, hi = c * NLOC, min((c + 1) * NLOC, N)
        sel = (dst0 >= lo) & (dst0 < hi)
        s, d = src0[sel], dst0[sel]
        o = np.argsort(d, kind="stable")
        per_core.append((s[o], d[o] - lo))

    cnt = np.zeros((NCORES, NWC), np.int64)
    for c in range(NCORES):
        _, dl = per_core[c]
        cnt[c] = np.bincount(dl // P, minlength=NWC)
    # balance: each core processes its windows in descending-count order so
    # the SPMD max-over-cores chunk padding aligns fat windows with fat ones
    orders = [np.argsort(-cnt[c], kind="stable") for c in range(NCORES)]
    lw_global = np.zeros(NCORES * NWC, np.int64)
    for c in range(NCORES):
        inv = np.empty(NWC, np.int64)
        inv[orders[c]] = np.arange(NWC)
        lw_global[c * NWC:(c + 1) * NWC] = inv
    s_cnt = np.stack([cnt[c][orders[c]] for c in range(NCORES)])
    # chunk 0 = self chunk; then real-edge chunks
    cw = 1 + np.ceil(s_cnt.max(axis=0) / P).astype(np.int64)    # [NWC]
    k0 = np.concatenate([[0], np.cumsum(cw)])
    K = int(k0[-1])
    CWMAX = int(cw.max())

    def rowid(n):
        # table row for node n in the AllGather slab layout:
        # core(n)*NLOC + partition(n)*NWC + processing_order_window(n)
        return (n // NLOC) * NLOC + (n % P) * NWC + lw_global[n // P]

    src_idx = np.zeros((NCORES, P, K), np.int32)
    dst_rel = np.zeros((NCORES, P, K), f16)
    ebp = np.full((NCORES, P, K), -150.0, np.float32)
    for c in range(NCORES):
        lo, hi = c * NLOC, min((c + 1) * NLOC, N)
        s, dl = per_core[c]
        w = dl // P
        starts = np.searchsorted(w, np.arange(NWC), side="left")
        ends = np.searchsorted(w, np.arange(NWC), side="right")
        for wi in range(NWC):
            g = int(orders[c][wi])
            # self chunk: slot p -> node p of window (real nodes only)
            nreal = max(0, min(hi - (lo + g * P), P))
            pr = np.arange(P)
            dst_rel[c, pr, k0[wi]] = pr.astype(f16)
            ebp[c, pr[:nreal], k0[wi]] = 0.0
            # real edges from chunk k0[wi]+1, src-sorted for DRAM locality
            a, b = int(starts[g]), int(ends[g])
            n = b - a
            sw = rowid(s[a:b])
            dw = dl[a:b] % P
            o2 = np.argsort(sw, kind="stable")
            sw, dw = sw[o2], dw[o2]
            j = np.arange(n)
            ch = k0[wi] + 1 + j // P
            pr = j % P
            src_idx[c, pr, ch] = sw
            dst_rel[c, pr, ch] = dw.astype(f16)
            ebp[c, pr, ch] = 0.0
            # pad edges spread over rel-slots
            slots = (int(cw[wi]) - 1) * P
            j = np.arange(n, slots)
            ch = k0[wi] + 1 + j // P
            pr = j % P
            src_idx[c, pr, ch] = 0
            dst_rel[c, pr, ch] = (j % P).astype(f16)

    # edge-major flat dst_rel for the broadcast-DMA transposed one-hot
    dst_flat = np.zeros((NCORES, 1, K * P), f16)
    for c in range(NCORES):
        dst_flat[c, 0, :] = dst_rel[c].T.reshape(-1)

    # x columns and graph masks follow each core's processing order
    xTloc = []
    for c in range(NCORES):
        xc3 = xT[:, c * NLOC:(c + 1) * NLOC].reshape(DIN, NWC, P)
        xTloc.append(np.ascontiguousarray(
            xc3[:, orders[c], :].reshape(DIN, NLOC)))

    gmask = np.zeros((NCORES, P, NWC * G), f16)
    for c in range(NCORES):
        lo, hi = c * NLOC, min((c + 1) * NLOC, N)
        for wi in range(NWC):
            g = int(orders[c][wi])
            nlo = lo + g * P
            nn = max(0, min(hi - nlo, P))
            if nn <= 0:
                continue
            gmask[c, np.arange(nn), wi * G + batch[nlo:nlo + nn]] = 1.0
    counts = np.bincount(batch, minlength=G).astype(np.float32)
    countsRecipB = np.tile(1.0 / np.maximum(counts, 1.0), (P, 1)).astype(np.float32)

    ebp *= S8  # e-values are S8-scaled on device; keep pad bias aligned

    meta = dict(cfg=cfg, NWC=NWC, NLOC=NLOC, NPAD=NPAD, K=K, CWMAX=CWMAX,
                cw=cw.tolist(), k0=k0.tolist(), Pp1=Pp1, Pp2=Pp2)
    data = dict(weights=weights, xTloc=xTloc, src_idx=src_idx,
                dst_rel=dst_rel, dst_flat=dst_flat, ebp=ebp, gmask=gmask,
                countsRecipB=countsRecipB)
    return meta, data


# ---------------------------------------------------------------------------
# device program
# ---------------------------------------------------------------------------

def build_program(meta):
    import concourse.bass as bass
    import concourse.bacc as bacc
    import concourse.tile as tile
    import concourse.mybir as mybir
    from concourse.masks import make_identity

    cfg = meta["cfg"]
    N, DIN, H, G, DOUT, NCORES = (cfg["N"], cfg["DIN"], cfg["H"], cfg["G"],
                                  cfg["DOUT"], cfg["NCORES"])
    NWC, NLOC, NPAD, K, CWMAX = (meta["NWC"], meta["NLOC"], meta["NPAD"],
                                 meta["K"], meta["CWMAX"])
    cw, k0 = meta["cw"], meta["k0"]
    NT = NPAD // P
    f32 = mybir.dt.float32
    f16 = mybir.dt.float16
    f8 = mybir.dt.float8e4
    AF = mybir.ActivationFunctionType
    OP = mybir.AluOpType

    nc = bacc.Bacc("TRN2", target_bir_lowering=False, debug=False,
                   num_devices=NCORES)

    d_xTloc = nc.dram_tensor("xTloc", [DIN, NLOC], f16, kind="ExternalInput")
    d_src = nc.dram_tensor("src_idx", [P, K], mybir.dt.int32, kind="ExternalInput")
    d_dst = nc.dram_tensor("dst_rel", [P, K], f16, kind="ExternalInput")
    d_dstf = nc.dram_tensor("dst_flat", [1, K * P], f16, kind="ExternalInput")
    d_ebp = nc.dram_tensor("ebp", [P, K], f32, kind="ExternalInput")
    d_gmask = nc.dram_tensor("gmask", [P, NWC * G], f16, kind="ExternalInput")
    d_crecip = nc.dram_tensor("countsRecipB", [P, G], f32, kind="ExternalInput")
    wnames = {
        "wnfc": ([DIN, H], f16), "bnfc": ([H, 1], f32),
        "wl1a": ([H, TW], f16), "wr1a": ([H, TW], f16),
        "bl1B": ([P, TW], f32), "br1B": ([P, TW], f32),
        "wl2a": ([H, TW], f16), "wr2a": ([H, TW], f16),
        "bl2B": ([P, TW], f32), "br2B": ([P, TW], f32),
        "arec1B": ([P, H], f32), "arec2B": ([P, H], f32),
        "bias1B": ([P, H], f32), "bias2B": ([P, H], f32),
        "wfc1": ([H, 32], f16), "bfc1": ([1, 32], f16),
        "wfc2": ([32, DOUT], f16), "bfc2": ([1, DOUT], f16),
    }
    d_w = {k: nc.dram_tensor(k, shp, dt, kind="ExternalInput")
           for k, (shp, dt) in wnames.items()}
    d_out = nc.dram_tensor("out", [G, DOUT], f32, kind="ExternalOutput")

    # fp8 tables assembled by AllGather of per-core xl slabs.
    # row for node n: core(n)*NLOC + (n%P)*NWC + local_window(n)
    d_tab1loc = nc.dram_tensor("tab1loc", [P, NWC * TW], f8, kind="Internal")
    d_tab2loc = nc.dram_tensor("tab2loc", [P, NWC * TW], f8, kind="Internal")
    d_tab1full = nc.dram_tensor("tab1full", [NPAD, TW], f8,
                                kind="Internal", addr_space="Shared")
    d_tab2full = nc.dram_tensor("tab2full", [NPAD, TW], f8,
                                kind="Internal", addr_space="Shared")
    d_gsin = nc.dram_tensor("gsin", [P, G], f32, kind="Internal")
    d_gsout = nc.dram_tensor("gsout", [P, G], f32, kind="Internal",
                             addr_space="Shared")

    def bcast_last(ap2d, c, j):
        return bass.AP(ap2d.tensor, ap2d.offset,
                       [list(ap2d.ap[0]), list(ap2d.ap[1]), [0, j]])

    def bcast_mid(ap2d, c):
        return bass.AP(ap2d.tensor, ap2d.offset,
                       [list(ap2d.ap[0]), [0, c], list(ap2d.ap[1])])

    def bcast_row(ap2d, e):
        # [P(=any), 1] -> [P, e] broadcast along free
        return bass.AP(ap2d.tensor, ap2d.offset,
                       [list(ap2d.ap[0]), [0, e]])

    def bcast_part(ap_row, n):
        # [1, E] dram row -> [n, E] partition broadcast
        return bass.AP(ap_row.tensor, ap_row.offset,
                       [[0, n], list(ap_row.ap[1])])

    with tile.TileContext(nc) as tc:
        with tc.tile_pool(name="const", bufs=1) as cpool:
            identf = cpool.tile([P, P], f32)
            make_identity(nc, identf[:, :])
            ident = cpool.tile([P, P], f16)
            nc.vector.tensor_copy(ident[:, :], identf[:, :])
            ident8 = cpool.tile([P, P], f8)
            nc.vector.tensor_copy(ident8[:, :], identf[:, :])
            iotaI = cpool.tile([P, P], mybir.dt.int32)
            nc.gpsimd.iota(iotaI[:, :], pattern=[[1, P]], base=0,
                           channel_multiplier=0)
            iotaF = cpool.tile([P, P], f16)
            nc.vector.tensor_copy(iotaF[:, :], iotaI[:, :])
            iotaPI = cpool.tile([P, 1], mybir.dt.int32)
            nc.gpsimd.iota(iotaPI[:, :], pattern=[[1, 1]], base=0,
                           channel_multiplier=1)
            iotaP = cpool.tile([P, 1], f16)
            nc.vector.tensor_copy(iotaP[:, :], iotaPI[:, :])
            ones1 = cpool.tile([1, P], f16)
            nc.vector.memset(ones1[:, :], 1.0)

            w_sb = {}
            for k, (shp, dt) in wnames.items():
                w_sb[k] = cpool.tile(shp, dt, name=f"w_{k}", tag=f"w_{k}")
                nc.sync.dma_start(out=w_sb[k][:, :], in_=d_w[k][:, :])
            src_sb = cpool.tile([P, K], mybir.dt.int32)
            nc.sync.dma_start(out=src_sb[:, :], in_=d_src[:, :])
            dst_sb = cpool.tile([P, K], f16)
            nc.sync.dma_start(out=dst_sb[:, :], in_=d_dst[:, :])
            ebp_sb = cpool.tile([P, K], f32)
            nc.sync.dma_start(out=ebp_sb[:, :], in_=d_ebp[:, :])
            crecip_sb = cpool.tile([P, G], f32)
            nc.sync.dma_start(out=crecip_sb[:, :], in_=d_crecip[:, :])

            with tc.tile_pool(name="big", bufs=1) as bigp:
                xr_sb = bigp.tile([P, NWC * TW], f16, tag="xr")
                xl_sb = bigp.tile([P, NWC * TW], f8, tag="xl")
                h2T_sb = bigp.tile([P, NLOC], f16, tag="h2T")

                # ====== table = AllGather of per-core xl slabs (fp8) ======
                def publish_table(d_tabloc, d_tabfull):
                    nc.sync.dma_start(out=d_tabloc[:, :], in_=xl_sb[:, :])
                    nc.gpsimd.collective_compute(
                        "AllGather", OP.bypass,
                        replica_groups=[list(range(NCORES))],
                        ins=[d_tabloc[:, :]],
                        outs=[d_tabfull[:, :].rearrange(
                            "(c r) f -> c (r f)", c=NCORES)])

                # local window products: xr (Wr) and xl (Wl, for self chunks)
                def build_loc(layer, hx_lhsT):
                    wra = w_sb["wr1a" if layer == 1 else "wr2a"]
                    brB = w_sb["br1B" if layer == 1 else "br2B"]
                    wla = w_sb["wl1a" if layer == 1 else "wl2a"]
                    blB = w_sb["bl1B" if layer == 1 else "bl2B"]
                    with (
                        tc.tile_pool(name=f"xr{layer}", bufs=3) as sp,
                        tc.tile_pool(name=f"xrps{layer}", bufs=2,
                                     space="PSUM") as pp,
                    ):
                        for w in range(NWC):
                            lhsT = hx_lhsT(w, sp, pp)
                            ps = pp.tile([P, TW], f32, tag="xr")
                            nc.tensor.matmul(out=ps[:, :], lhsT=lhsT,
                                             rhs=wra[:, :], start=True, stop=True)
                            nc.vector.scalar_tensor_tensor(
                                out=xr_sb[:, w * TW:(w + 1) * TW], in0=ps[:, :],
                                scalar=S8, in1=brB[:, :],
                                op0=OP.mult, op1=OP.add)
                            ps2 = pp.tile([P, TW], f32, tag="xl")
                            nc.tensor.matmul(out=ps2[:, :], lhsT=lhsT,
                                             rhs=wla[:, :], start=True, stop=True)
                            nc.vector.scalar_tensor_tensor(
                                out=xl_sb[:, w * TW:(w + 1) * TW], in0=ps2[:, :],
                                scalar=S8, in1=blB[:, :],
                                op0=OP.mult, op1=OP.add)

                lhx_cache = {}

                def l1_loc_lhsT(w, sp, pp):
                    gidx = w // 4
                    if gidx not in lhx_cache:
                        g0 = gidx * 512
                        gl = min(512, NLOC - g0)
                        xg = sp.tile([DIN, 512], f16, tag="xgl")
                        nc.sync.dma_start(out=xg[:, :gl],
                                          in_=d_xTloc[:, g0:g0 + gl])
                        psn = pp.tile([P, 512], f32, tag="nfcl")
                        nc.tensor.matmul(out=psn[:, :gl],
                                         lhsT=w_sb["wnfc"][:, :],
                                         rhs=xg[:, :gl], start=True, stop=True)
                        hxg = sp.tile([P, 512], f16, tag="hxgl")
                        nc.scalar.activation(out=hxg[:, :gl], in_=psn[:, :gl],
                                             func=AF.Lrelu,
                                             bias=w_sb["bnfc"][:, :],
                                             scale=1.0, alpha=0.01)
                        lhx_cache.clear()
                        lhx_cache[gidx] = hxg
                    s = (w % 4) * P
                    return lhx_cache[gidx][:, s:s + P]

                build_loc(1, l1_loc_lhsT)
                lhx_cache.clear()
                publish_table(d_tab1loc, d_tab1full)

                # ============ edge phase ============
                def edge_phase(layer, d_tab, Pp, h_out_cb):
                    arecB = w_sb["arec1B" if layer == 1 else "arec2B"]
                    biasB = w_sb["bias1B" if layer == 1 else "bias2B"]
                    GW = CWMAX * TW
                    with (
                        tc.tile_pool(name=f"eg{layer}", bufs=4) as gp,
                        tc.tile_pool(name=f"ew{layer}", bufs=3) as sp,
                        tc.tile_pool(name=f"es{layer}", bufs=4) as ssp,
                        tc.tile_pool(name=f"eps{layer}", bufs=4,
                                     space="PSUM") as ppm,
                        tc.tile_pool(name=f"epo{layer}", bufs=2,
                                     space="PSUM") as ppo,
                    ):
                        for w in range(NWC):
                            c = cw[w]
                            ks = k0[w]
                            ng = c - 1
                            xlw = xl_sb[:, w * TW:(w + 1) * TW]
                            xrw = xr_sb[:, w * TW:(w + 1) * TW]
                            # contiguous window tile: [xl_window | gathered...]
                            gt = gp.tile([P, GW], f8, tag="g")
                            nc.vector.tensor_copy(gt[:, 0:TW], xlw)
                            for j in range(1, c):
                                nc.gpsimd.indirect_dma_start(
                                    out=gt[:, j * TW:(j + 1) * TW],
                                    out_offset=None,
                                    in_=d_tab[:, :],
                                    in_offset=bass.IndirectOffsetOnAxis(
                                        ap=src_sb[:, ks + j:ks + j + 1],
                                        axis=0))

                            def rhs(j):
                                return gt[:, j * TW:(j + 1) * TW]

                            # edge-partitioned one-hot [P, c, 128]
                            oh = sp.tile([P, CWMAX * P], f16, tag="oh")
                            oh3 = oh[:, :c * P].rearrange("p (c j) -> p c j", j=P)
                            nc.vector.tensor_tensor(
                                out=oh3,
                                in0=bcast_last(dst_sb[:, ks:ks + c], c, P),
                                in1=bcast_mid(iotaF[:, :], c),
                                op=OP.is_equal)
                            # transposed one-hot via partition-broadcast DMA
                            dstb = sp.tile([P, CWMAX * P], f16, tag="dstb")
                            nc.sync.dma_start(
                                out=dstb[:, :c * P],
                                in_=bcast_part(d_dstf[:, ks * P:(ks + c) * P], P))
                            ohT = sp.tile([P, CWMAX * P], f16, tag="ohT")
                            nc.vector.tensor_tensor(
                                out=ohT[:, :c * P],
                                in0=bcast_row(iotaP[:, :], c * P),
                                in1=dstb[:, :c * P], op=OP.is_equal)

                            # m' in 3-chunk psum groups (one bank each)
                            e2w = ssp.tile([P, CWMAX], f32, tag="e2w")
                            for g0 in range(0, c, 3):
                                hc = min(3, c - g0)
                                psm = ppm.tile([P, 3 * TW], f32, tag="m")
                                psm3 = psm[:, :].rearrange(
                                    "p (t f) -> p t f", f=TW)
                                nc.tensor.matmul(
                                    out=psm[:, 0:hc * TW], lhsT=ident8[:, :],
                                    rhs=gt[:, g0 * TW:(g0 + hc) * TW],
                                    start=True, stop=False)
                                for jj in range(hc):
                                    j = g0 + jj
                                    nc.tensor.matmul(
                                        out=psm3[:, jj, 0:TW],
                                        lhsT=ohT[:, j * P:(j + 1) * P],
                                        rhs=xrw, start=False, stop=True)
                                rp = ssp.tile([P, 4], f32, tag="rp")
                                nc.vector.tensor_reduce(
                                    out=rp[:, :hc], in_=psm3[:, 0:hc, 0:Pp],
                                    axis=mybir.AxisListType.X, op=OP.add,
                                    apply_absolute_value=True)
                                e0 = ssp.tile([P, 4], f32, tag="e0")
                                if Pp < H:
                                    rn = ssp.tile([P, 4], f32, tag="rn")
                                    nc.vector.tensor_reduce(
                                        out=rn[:, :hc], in_=psm3[:, 0:hc, Pp:H],
                                        axis=mybir.AxisListType.X, op=OP.add,
                                        apply_absolute_value=True)
                                    nc.vector.tensor_tensor(
                                        out=e0[:, :hc], in0=rp[:, :hc],
                                        in1=rn[:, :hc], op=OP.subtract)
                                else:
                                    nc.vector.tensor_copy(e0[:, :hc], rp[:, :hc])
                                e1 = ssp.tile([P, 4], f32, tag="e1")
                                nc.vector.scalar_tensor_tensor(
                                    out=e1[:, :hc], in0=psm3[:, 0:hc, H],
                                    scalar=1.5, in1=e0[:, :hc],
                                    op0=OP.mult, op1=OP.add)
                                nc.vector.tensor_tensor(
                                    out=e2w[:, g0:g0 + hc], in0=e1[:, :hc],
                                    in1=ebp_sb[:, ks + g0:ks + g0 + hc],
                                    op=OP.add)
                            # one Exp per window
                            av = ssp.tile([P, CWMAX], f32, tag="av")
                            nc.scalar.activation(
                                out=av[:, :c], in_=e2w[:, :c],
                                func=AF.Exp, scale=0.4 / S8)

                            # av-scaled rows (fp8 -> f16); one-hot stays f16
                            pso = ppo.tile([P, H + 2], f32, tag="out")
                            for j in range(c):
                                avg = ssp.tile([P, TW], f16, tag="avg", bufs=8)
                                nc.vector.tensor_scalar(
                                    out=avg[:, 0:H + 2],
                                    in0=rhs(j)[:, 0:H + 2],
                                    scalar1=av[:, j:j + 1], scalar2=None,
                                    op0=OP.mult)
                                nc.tensor.matmul(
                                    out=pso[:, :], lhsT=oh[:, j * P:(j + 1) * P],
                                    rhs=avg[:, 0:H + 2],
                                    start=(j == 0), stop=(j == c - 1))
                            dcl = ssp.tile([P, 1], f32, tag="dcl")
                            nc.vector.tensor_scalar_max(dcl[:, :],
                                                        pso[:, H + 1:H + 2],
                                                        1e-20)
                            rd = ssp.tile([P, 1], f32, tag="rd")
                            nc.vector.reciprocal(rd[:, :], dcl[:, :])
                            h1 = ssp.tile([P, H], f32, tag="h1")
                            nc.vector.scalar_tensor_tensor(
                                out=h1[:, :], in0=pso[:, 0:H], scalar=rd[:, :],
                                in1=arecB[:, :], op0=OP.mult, op1=OP.mult)
                            h2 = ssp.tile([P, H], f32, tag="h2")
                            nc.vector.tensor_tensor(
                                out=h2[:, :], in0=h1[:, :], in1=biasB[:, :],
                                op=OP.add)
                            # exact leaky_relu(0.01) on the vector engine:
                            # max(z,0) + 0.01*min(z,0)
                            hwp = ssp.tile([P, H], f32, tag="hwp")
                            nc.vector.tensor_scalar_max(hwp[:, :], h2[:, :], 0.0)
                            hwn = ssp.tile([P, H], f32, tag="hwn")
                            nc.vector.tensor_scalar_min(hwn[:, :], h2[:, :], 0.0)
                            hw_ = ssp.tile([P, H], f16, tag="hw")
                            nc.vector.scalar_tensor_tensor(
                                out=hw_[:, :], in0=hwn[:, :], scalar=0.01,
                                in1=hwp[:, :], op0=OP.mult, op1=OP.add)
                            h_out_cb(w, hw_, ssp, ppo)

                def l1_out(w, hw_, ssp, ppt):
                    psT = ppt.tile([P, P], f16, tag="tr")
                    nc.tensor.transpose(out=psT[:, :], in_=hw_[:, :],
                                        identity=ident[:, :])
                    nc.vector.tensor_copy(h2T_sb[:, w * P:(w + 1) * P],
                                          psT[:, :])

                edge_phase(1, d_tab1full, meta["Pp1"], l1_out)

                # ---- layer-2 locals + table (from h2T, no h2 exchange) ----
                def l2_loc_lhsT(w, sp, pp):
                    return h2T_sb[:, w * P:(w + 1) * P]

                build_loc(2, l2_loc_lhsT)
                publish_table(d_tab2loc, d_tab2full)

                # ---- layer 2 edge phase + pooling accumulate ----
                with (
                    tc.tile_pool(name="gm", bufs=3) as gmp,
                    tc.tile_pool(name="gps", bufs=1, space="PSUM") as gpsp,
                ):
                    ps_gs = gpsp.tile([P, G], f32, tag="gs")

                    def l2_out(w, hw_, ssp, ppt):
                        gm = gmp.tile([P, G], f16, tag="gm")
                        nc.sync.dma_start(out=gm[:, :],
                                          in_=d_gmask[:, w * G:(w + 1) * G])
                        nc.tensor.matmul(out=ps_gs[:, :], lhsT=hw_[:, :],
                                         rhs=gm[:, :], start=(w == 0),
                                         stop=(w == NWC - 1))

                    edge_phase(2, d_tab2full, meta["Pp2"], l2_out)

                    with (
                        tc.tile_pool(name="fc", bufs=1) as fp,
                        tc.tile_pool(name="fcps", bufs=1, space="PSUM") as fpp,
                    ):
                        gsum = fp.tile([P, G], f32)
                        nc.vector.tensor_copy(gsum[:, :], ps_gs[:, :])
                        nc.sync.dma_start(out=d_gsin[:, :], in_=gsum[:, :])
                        nc.gpsimd.collective_compute(
                            "AllReduce", OP.add,
                            replica_groups=[list(range(NCORES))],
                            ins=[d_gsin[:, :]], outs=[d_gsout[:, :]])
                        gsum2 = fp.tile([P, G], f32)
                        nc.sync.dma_start(out=gsum2[:, :], in_=d_gsout[:, :])
                        meanT = fp.tile([P, G], f16)
                        nc.vector.tensor_tensor(out=meanT[:, :], in0=gsum2[:, :],
                                                in1=crecip_sb[:, :], op=OP.mult)
                        psf = fpp.tile([P, 32], f32, tag="f1")
                        nc.tensor.matmul(out=psf[:G, :], lhsT=meanT[:, :G],
                                         rhs=w_sb["wfc1"][:, :],
                                         start=True, stop=False)
                        nc.tensor.matmul(out=psf[:G, :], lhsT=ones1[:, :G],
                                         rhs=w_sb["bfc1"][:, :],
                                         start=False, stop=True)
                        hf1 = fp.tile([P, 32], f16)
                        nc.scalar.activation(out=hf1[:G, :], in_=psf[:G, :],
                                             func=AF.Lrelu, alpha=0.01)
                        psT = fpp.tile([P, P], f16, tag="ft")
                        nc.tensor.transpose(out=psT[:32, :G], in_=hf1[:G, :32],
                                            identity=ident[:G, :G])
                        hf1T = fp.tile([32, P], f16)
                        nc.scalar.activation(out=hf1T[:, :G], in_=psT[:32, :G],
                                             func=AF.Copy)
                        pso = fpp.tile([P, DOUT], f32, tag="f2")
                        nc.tensor.matmul(out=pso[:G, :], lhsT=hf1T[:, :G],
                                         rhs=w_sb["wfc2"][:, :],
                                         start=True, stop=False)
                        nc.tensor.matmul(out=pso[:G, :], lhsT=ones1[:, :G],
                                         rhs=w_sb["bfc2"][:, :],
                                         start=False, stop=True)
                        fout = fp.tile([P, DOUT], f32)
                        nc.vector.tensor_copy(fout[:G, :], pso[:G, :])
                        nc.sync.dma_start(out=d_out[:, :], in_=fout[:G, :])

    nc.compile()
    return nc


# ---------------------------------------------------------------------------
# runner
# ---------------------------------------------------------------------------

def _in_maps(meta, data):
    cfg = meta["cfg"]
    maps = []
    for c in range(cfg["NCORES"]):
        m = {
            "xTloc": data["xTloc"][c],
            "src_idx": data["src_idx"][c],
            "dst_rel": data["dst_rel"][c],
            "dst_flat": data["dst_flat"][c],
            "ebp": data["ebp"][c],
            "gmask": data["gmask"][c],
            "countsRecipB": data["countsRecipB"],
        }
        for k, v in data["weights"].items():
            m[k] = np.ascontiguousarray(v)
        maps.append(m)
    return maps


def run_on_device(inputs, cfg, trace=False):
    from concourse.bass_utils import run_bass_kernel_spmd
    meta, data = host_prep(inputs, cfg)
    nc = build_program(meta)
    res = run_bass_kernel_spmd(nc, _in_maps(meta, data),
                               core_ids=list(range(cfg["NCORES"])), trace=trace)
    return res


def kernel(**inputs):
    res = run_on_device(inputs, FULL_CFG, trace=False)
    return np.asarray(res.results[0]["out"], np.float32)
